# revision 12
# baseline (speedup 1.0000x reference)
"""Trainium2 Bass kernel for nn_Attention (B=64, S=2048, RNN=1024, ATT_HID=512).

Data-parallel over batch across 8 NeuronCores. Each core handles 8 batches:
  att_h  = h @ W_h.T + b_h                     (PE, setup, fp32)
  scores = w_a . tanh(p_att + att_h)           (DVE add + ACT tanh + DVE fused mul-reduce)
  wexp   = mask * exp(scores)                  (ACT exp + DVE fused mul-reduce -> row sums)
  out    = (sum_s wexp[s] * att_feats[s]) / sum_s wexp[s]   (PE matmuls + ACT copy-scale)

The softmax -> mask -> renormalize of the reference reduces algebraically to
mask*exp(s) / sum(mask*exp(s)); scores are O(1) so exp needs no max-subtraction.

The two big streams (p_att_feats, att_feats) are converted to bf16 on the host:
halves HBM traffic (the bottleneck), avoids the 2-pass fp32 matmul split on PE,
and doubles DVE throughput. Score accumulation stays fp32 (STT accum + exp), so
the only bf16 rounding is on tanh inputs/outputs and the weighted-feature sum;
measured end-to-end relative error ~1e-3 vs the fp32 reference.
"""

import sys

import numpy as np

for _p in ("/opt/trn_rl_repo",):
    if _p not in sys.path:
        sys.path.append(_p)

from contextlib import ExitStack

import ml_dtypes

import concourse.bass as bass
from concourse import bacc, mybir, tile
from concourse.bass import ts
from concourse.bass_utils import run_bass_kernel_spmd

B, S, RNN, HID = 64, 2048, 1024, 512
N_CORES = 8
BL = B // N_CORES

USE_BF16 = True
DT_NP = ml_dtypes.bfloat16 if USE_BF16 else np.float32


def build_nc(BL=BL, S=S, RNN=RNN, HID=HID, n_cores=N_CORES, use_bf16=USE_BF16):
    P = 128
    NT = S // P            # score chunks of 128 positions
    KC = RNN // P          # contraction chunks for att_h matmul
    NH = max(1, RNN // 512)  # output column blocks (matmul N<=512)
    HW = RNN // NH
    CP = min(16, NT)       # s-chunks per p-DMA
    CF = min(8, NT)        # s-chunks per f-DMA
    NJP = NT // CP
    NJF = NT // CF
    f32 = mybir.dt.float32
    dt = mybir.dt.bfloat16 if use_bf16 else f32
    Act = mybir.ActivationFunctionType
    Alu = mybir.AluOpType

    nc = bacc.Bacc(
        "TRN2",
        target_bir_lowering=False,
        debug=False,
        enable_asserts=False,
        num_devices=n_cores,
    )

    p_t = nc.dram_tensor("p", [BL, S, HID], dt, kind="ExternalInput").ap()
    f_t = nc.dram_tensor("f", [BL, S, RNN], dt, kind="ExternalInput").ap()
    hT_t = nc.dram_tensor("hT", [RNN, BL], f32, kind="ExternalInput").ap()
    WhT_t = nc.dram_tensor("WhT", [RNN, HID], f32, kind="ExternalInput").ap()
    bh_t = nc.dram_tensor("bh", [1, HID], f32, kind="ExternalInput").ap()
    wa_t = nc.dram_tensor("wa", [1, HID], f32, kind="ExternalInput").ap()
    mk_t = nc.dram_tensor("maskc", [BL, P, NT], f32, kind="ExternalInput").ap()
    out_t = nc.dram_tensor("out", [BL, RNN], f32, kind="ExternalOutput").ap()

    with tile.TileContext(nc) as tc, ExitStack() as ctx:
        const = ctx.enter_context(tc.tile_pool(name="const", bufs=1))

        WhT_sb = const.tile([P, KC * HID], f32, tag="WhT")
        nc.sync.dma_start(
            WhT_sb.rearrange("p (c n) -> p c n", c=KC),
            WhT_t.rearrange("(c p) n -> p c n", p=P),
        )
        hT_sb = const.tile([P, KC * BL], f32, tag="hT")
        nc.sync.dma_start(
            hT_sb.rearrange("p (c b) -> p c b", c=KC),
            hT_t.rearrange("(c p) b -> p c b", p=P),
        )
        bh_sb = const.tile([1, HID], f32, tag="bh")
        nc.sync.dma_start(bh_sb, bh_t)
        wa_sb = const.tile([1, HID], f32, tag="wa")
        nc.sync.dma_start(wa_sb, wa_t)
        mask_sb = const.tile([P, BL * NT], f32, tag="mask")
        nc.sync.dma_start(
            mask_sb.rearrange("p (b t) -> p b t", b=BL),
            mk_t.rearrange("b p t -> p b t"),
        )
        ones_row = const.tile([1, P], f32, tag="ones_row")
        nc.vector.memset(ones_row, 1.0)
        ones_bl = const.tile([1, BL], f32, tag="ones_bl")
        nc.vector.memset(ones_bl, 1.0)
        ones_col = const.tile([P, 1], f32, tag="ones_col")
        nc.vector.memset(ones_col, 1.0)
        att_rows = const.tile([1, BL * HID], f32, tag="att_rows")
        att_h_sb = const.tile([BL, HID], f32, tag="att_h")
        wab_sb = const.tile([P, HID], dt, tag="wab")

        with tc.tile_pool(name="ps_setup", bufs=1, space="PSUM") as pss:
            ah_ps = pss.tile([BL, HID], f32, tag="ah")
            for c in range(KC):
                nc.tensor.matmul(
                    ah_ps,
                    hT_sb[:, ts(c, BL)],
                    WhT_sb[:, ts(c, HID)],
                    start=(c == 0),
                    stop=False,
                )
            nc.tensor.matmul(ah_ps, ones_bl, bh_sb, start=False, stop=True)
            nc.vector.tensor_copy(att_h_sb, ah_ps)
            wab_ps = pss.tile([P, HID], f32, tag="wab_ps")
            nc.tensor.matmul(wab_ps, ones_row, wa_sb, start=True, stop=True)
            nc.scalar.copy(wab_sb, wab_ps)

        # att_h rows staged to partition 0 so the per-batch broadcast matmul
        # has a partition-0 rhs
        for b in range(BL):
            nc.sync.dma_start(att_rows[:, ts(b, HID)], att_h_sb[b : b + 1, :])

        ps_bc = ctx.enter_context(tc.tile_pool(name="ps_bc", bufs=2, space="PSUM"))
        ps_o = ctx.enter_context(tc.tile_pool(name="ps_o", bufs=2, space="PSUM"))
        pp = ctx.enter_context(tc.tile_pool(name="pp", bufs=2))
        py = ctx.enter_context(tc.tile_pool(name="py", bufs=2))
        pf = ctx.enter_context(tc.tile_pool(name="pf", bufs=4))
        psc = ctx.enter_context(tc.tile_pool(name="psc", bufs=2))
        pah = ctx.enter_context(tc.tile_pool(name="pah", bufs=2))
        pout = ctx.enter_context(tc.tile_pool(name="pout", bufs=2))

        for b in range(BL):
            bc_ps = ps_bc.tile([P, HID], f32, tag="bc")
            nc.tensor.matmul(
                bc_ps, ones_row, att_rows[:, ts(b, HID)], start=True, stop=True
            )
            ahb = pah.tile([P, HID], dt, tag="ahb")
            nc.scalar.copy(ahb, bc_ps)

            s_all = psc.tile([P, NT], f32, tag="s")
            fts = []
            for j in range(NJP):
                pt = pp.tile([P, CP * HID], dt, tag="p")
                nc.sync.dma_start(
                    pt.rearrange("p (i h) -> p i h", i=CP),
                    p_t[b, ts(j, CP * P), :].rearrange("(i p) h -> p i h", p=P),
                )
                nc.gpsimd.tensor_add(
                    pt.rearrange("p (i h) -> p i h", i=CP),
                    pt.rearrange("p (i h) -> p i h", i=CP),
                    ahb[:, None, :].broadcast_to([P, CP, HID]),
                )
                yt = py.tile([P, CP * HID], dt, tag="y")
                nc.scalar.activation(yt, pt, Act.Tanh)
                for i in range(CP):
                    t_idx = j * CP + i
                    nc.vector.scalar_tensor_tensor(
                        out=pt[:, ts(i, HID)],
                        in0=yt[:, ts(i, HID)],
                        scalar=1.0,
                        in1=wab_sb,
                        op0=Alu.mult,
                        op1=Alu.mult,
                        accum_out=s_all[:, t_idx : t_idx + 1],
                    )
                # interleave att_feats loads with score compute
                for jf in range(j * NJF // NJP, (j + 1) * NJF // NJP):
                    ft = pf.tile([P, CF * RNN], dt, tag="f")
                    nc.sync.dma_start(
                        ft.rearrange("p (i d) -> p i d", i=CF),
                        f_t[b, ts(jf, CF * P), :].rearrange("(i p) d -> p i d", p=P),
                    )
                    fts.append(ft)

            e_all = psc.tile([P, NT], f32, tag="e")
            nc.scalar.activation(e_all, s_all, Act.Exp)
            w_all = psc.tile([P, NT], dt, tag="w")
            rowsum = psc.tile([P, 1], f32, tag="rs")
            nc.vector.scalar_tensor_tensor(
                out=w_all,
                in0=e_all,
                scalar=1.0,
                in1=mask_sb[:, ts(b, NT)],
                op0=Alu.mult,
                op1=Alu.mult,
                accum_out=rowsum,
            )
            den_ps = ps_o.tile([1, 1], f32, tag="den")
            nc.tensor.matmul(den_ps, rowsum, ones_col, start=True, stop=True)
            rden = psc.tile([1, 1], f32, tag="rden")
            nc.vector.reciprocal(rden, den_ps)

            ohs = [
                ps_o.tile([1, HW], f32, tag=f"o{h}", name=f"oh{h}") for h in range(NH)
            ]
            for t in range(NT):
                ft = fts[t // CF]
                ibase = (t % CF) * RNN
                for h in range(NH):
                    nc.tensor.matmul(
                        ohs[h],
                        w_all[:, t : t + 1],
                        ft[:, ibase + h * HW : ibase + (h + 1) * HW],
                        start=(t == 0),
                        stop=(t == NT - 1),
                    )
            out_sb = pout.tile([1, RNN], f32, tag="outrow")
            for h in range(NH):
                nc.scalar.activation(
                    out_sb[:, ts(h, HW)], ohs[h], Act.Copy, scale=rden
                )
            nc.sync.dma_start(out_t[b : b + 1, :], out_sb)

    nc.compile()
    return nc


_NC_CACHE = {}


def _get_nc():
    if "nc" not in _NC_CACHE:
        _NC_CACHE["nc"] = build_nc()
    return _NC_CACHE["nc"]


def build_in_maps(h, att_feats, p_att_feats, att_masks, W_h, b_h, w_a):
    h = np.asarray(h, dtype=np.float32)
    W_h = np.asarray(W_h, dtype=np.float32)
    b_h = np.asarray(b_h, dtype=np.float32)
    w_a = np.asarray(w_a, dtype=np.float32)
    NT = S // 128
    WhT = np.ascontiguousarray(W_h.T)
    bh = b_h.reshape(1, HID)
    wa = w_a.reshape(1, HID)
    in_maps = []
    for c in range(N_CORES):
        sl = slice(c * BL, (c + 1) * BL)
        mc = (
            np.asarray(att_masks[sl])
            .astype(np.float32)
            .reshape(BL, NT, 128)
            .transpose(0, 2, 1)
        )
        in_maps.append(
            {
                "p": np.ascontiguousarray(np.asarray(p_att_feats[sl]).astype(DT_NP)),
                "f": np.ascontiguousarray(np.asarray(att_feats[sl]).astype(DT_NP)),
                "hT": np.ascontiguousarray(h[sl].T),
                "WhT": WhT,
                "bh": bh,
                "wa": wa,
                "maskc": np.ascontiguousarray(mc),
            }
        )
    return in_maps


def run(in_maps, trace=False, **kwargs):
    nc = _get_nc()
    return run_bass_kernel_spmd(
        nc, in_maps, core_ids=list(range(N_CORES)), trace=trace, **kwargs
    )


def kernel(h, att_feats, p_att_feats, att_masks, W_h, b_h, w_a, b_a=None):
    # b_a shifts every score equally; softmax normalization cancels it.
    in_maps = build_in_maps(h, att_feats, p_att_feats, att_masks, W_h, b_h, w_a)
    res = run(in_maps, trace=False)
    return np.concatenate([r["out"] for r in res.results], axis=0)


# revision 13
# speedup vs baseline: 1.3883x; 1.3883x over previous
"""Trainium2 Bass kernel for nn_Attention (B=64, S=2048, RNN=1024, ATT_HID=512).

Data-parallel over batch across 8 NeuronCores. Each core handles 8 batches:
  att_h  = h @ W_h.T + b_h                     (PE, setup, fp32)
  scores = w_a . tanh(p_att + att_h)           (DVE add + ACT tanh + DVE fused mul-reduce)
  wexp   = mask * exp(scores)                  (ACT exp + DVE fused mul-reduce -> row sums)
  out    = (sum_s wexp[s] * att_feats[s]) / sum_s wexp[s]   (PE matmuls + ACT copy-scale)

The softmax -> mask -> renormalize of the reference reduces algebraically to
mask*exp(s) / sum(mask*exp(s)); scores are O(1) so exp needs no max-subtraction.

The two big streams (p_att_feats, att_feats) are converted to bf16 on the host:
halves HBM traffic (the bottleneck), avoids the 2-pass fp32 matmul split on PE,
and doubles DVE throughput. Score accumulation stays fp32 (STT accum + exp), so
the only bf16 rounding is on tanh inputs/outputs and the weighted-feature sum;
measured end-to-end relative error ~1e-3 vs the fp32 reference.
"""

import sys

import numpy as np

for _p in ("/opt/trn_rl_repo",):
    if _p not in sys.path:
        sys.path.append(_p)

from contextlib import ExitStack

import ml_dtypes

import concourse.bass as bass
from concourse import bacc, mybir, tile
from concourse.bass import ts
from concourse.bass_utils import run_bass_kernel_spmd

B, S, RNN, HID = 64, 2048, 1024, 512
N_CORES = 8
BL = B // N_CORES

USE_BF16 = True
DT_NP = ml_dtypes.bfloat16 if USE_BF16 else np.float32


def build_nc(BL=BL, S=S, RNN=RNN, HID=HID, n_cores=N_CORES, use_bf16=USE_BF16):
    P = 128
    NT = S // P            # score chunks of 128 positions
    KC = RNN // P          # contraction chunks for att_h matmul
    NH = max(1, RNN // 512)  # output column blocks (matmul N<=512)
    HW = RNN // NH
    CP = min(16, NT)       # s-chunks per p-DMA
    CF = min(8, NT)        # s-chunks per f-DMA
    NJP = NT // CP
    NJF = NT // CF
    f32 = mybir.dt.float32
    dt = mybir.dt.bfloat16 if use_bf16 else f32
    Act = mybir.ActivationFunctionType
    Alu = mybir.AluOpType

    nc = bacc.Bacc(
        "TRN2",
        target_bir_lowering=False,
        debug=False,
        enable_asserts=False,
        num_devices=n_cores,
    )

    p_t = nc.dram_tensor("p", [BL, S, HID], dt, kind="ExternalInput").ap()
    f_t = nc.dram_tensor("f", [BL, S, RNN], dt, kind="ExternalInput").ap()
    hT_t = nc.dram_tensor("hT", [RNN, BL], f32, kind="ExternalInput").ap()
    WhT_t = nc.dram_tensor("WhT", [RNN, HID], f32, kind="ExternalInput").ap()
    bh_t = nc.dram_tensor("bh", [1, HID], f32, kind="ExternalInput").ap()
    wa_t = nc.dram_tensor("wa", [1, HID], f32, kind="ExternalInput").ap()
    mk_t = nc.dram_tensor("maskc", [BL, P, NT], f32, kind="ExternalInput").ap()
    out_t = nc.dram_tensor("out", [BL, RNN], f32, kind="ExternalOutput").ap()

    with tile.TileContext(nc) as tc, ExitStack() as ctx:
        const = ctx.enter_context(tc.tile_pool(name="const", bufs=1))

        WhT_sb = const.tile([P, KC * HID], f32, tag="WhT")
        nc.sync.dma_start(
            WhT_sb.rearrange("p (c n) -> p c n", c=KC),
            WhT_t.rearrange("(c p) n -> p c n", p=P),
        )
        hT_sb = const.tile([P, KC * BL], f32, tag="hT")
        nc.sync.dma_start(
            hT_sb.rearrange("p (c b) -> p c b", c=KC),
            hT_t.rearrange("(c p) b -> p c b", p=P),
        )
        bh_sb = const.tile([1, HID], f32, tag="bh")
        nc.sync.dma_start(bh_sb, bh_t)
        wa_sb = const.tile([1, HID], f32, tag="wa")
        nc.sync.dma_start(wa_sb, wa_t)
        mask_sb = const.tile([P, BL * NT], f32, tag="mask")
        nc.sync.dma_start(
            mask_sb.rearrange("p (b t) -> p b t", b=BL),
            mk_t.rearrange("b p t -> p b t"),
        )
        ones_row = const.tile([1, P], f32, tag="ones_row")
        nc.vector.memset(ones_row, 1.0)
        ones_bl = const.tile([1, BL], f32, tag="ones_bl")
        nc.vector.memset(ones_bl, 1.0)
        ones_col = const.tile([P, 1], f32, tag="ones_col")
        nc.vector.memset(ones_col, 1.0)
        att_rows = const.tile([1, BL * HID], f32, tag="att_rows")
        att_h_sb = const.tile([BL, HID], f32, tag="att_h")
        wab_sb = const.tile([P, HID], dt, tag="wab")

        with tc.tile_pool(name="ps_setup", bufs=1, space="PSUM") as pss:
            ah_ps = pss.tile([BL, HID], f32, tag="ah")
            for c in range(KC):
                nc.tensor.matmul(
                    ah_ps,
                    hT_sb[:, ts(c, BL)],
                    WhT_sb[:, ts(c, HID)],
                    start=(c == 0),
                    stop=False,
                )
            nc.tensor.matmul(ah_ps, ones_bl, bh_sb, start=False, stop=True)
            nc.vector.tensor_copy(att_h_sb, ah_ps)
            wab_ps = pss.tile([P, HID], f32, tag="wab_ps")
            nc.tensor.matmul(wab_ps, ones_row, wa_sb, start=True, stop=True)
            nc.scalar.copy(wab_sb, wab_ps)

        # att_h rows staged to partition 0 so the per-batch broadcast matmul
        # has a partition-0 rhs
        for b in range(BL):
            nc.sync.dma_start(att_rows[:, ts(b, HID)], att_h_sb[b : b + 1, :])

        ps_bc = ctx.enter_context(tc.tile_pool(name="ps_bc", bufs=2, space="PSUM"))
        ps_o = ctx.enter_context(tc.tile_pool(name="ps_o", bufs=2, space="PSUM"))
        pp = ctx.enter_context(tc.tile_pool(name="pp", bufs=2))
        py = ctx.enter_context(tc.tile_pool(name="py", bufs=2))
        pf = ctx.enter_context(tc.tile_pool(name="pf", bufs=4))
        psc = ctx.enter_context(tc.tile_pool(name="psc", bufs=2))
        pah = ctx.enter_context(tc.tile_pool(name="pah", bufs=2))
        pout = ctx.enter_context(tc.tile_pool(name="pout", bufs=2))

        for b in range(BL):
            bc_ps = ps_bc.tile([P, HID], f32, tag="bc")
            nc.tensor.matmul(
                bc_ps, ones_row, att_rows[:, ts(b, HID)], start=True, stop=True
            )
            ahb = pah.tile([P, HID], dt, tag="ahb")
            nc.scalar.copy(ahb, bc_ps)

            s_all = psc.tile([P, NT], f32, tag="s")
            fts = []
            for j in range(NJP):
                pt = pp.tile([P, CP * HID], dt, tag="p")
                nc.sync.dma_start(
                    pt.rearrange("p (i h) -> p i h", i=CP),
                    p_t[b, ts(j, CP * P), :].rearrange("(i p) h -> p i h", p=P),
                )
                nc.vector.tensor_add(
                    pt.rearrange("p (i h) -> p i h", i=CP),
                    pt.rearrange("p (i h) -> p i h", i=CP),
                    ahb[:, None, :].broadcast_to([P, CP, HID]),
                )
                yt = py.tile([P, CP * HID], dt, tag="y")
                nc.scalar.activation(yt, pt, Act.Tanh)
                for i in range(CP):
                    t_idx = j * CP + i
                    nc.vector.scalar_tensor_tensor(
                        out=pt[:, ts(i, HID)],
                        in0=yt[:, ts(i, HID)],
                        scalar=1.0,
                        in1=wab_sb,
                        op0=Alu.mult,
                        op1=Alu.mult,
                        accum_out=s_all[:, t_idx : t_idx + 1],
                    )
                # interleave att_feats loads with score compute
                for jf in range(j * NJF // NJP, (j + 1) * NJF // NJP):
                    ft = pf.tile([P, CF * RNN], dt, tag="f")
                    nc.sync.dma_start(
                        ft.rearrange("p (i d) -> p i d", i=CF),
                        f_t[b, ts(jf, CF * P), :].rearrange("(i p) d -> p i d", p=P),
                    )
                    fts.append(ft)

            e_all = psc.tile([P, NT], f32, tag="e")
            nc.scalar.activation(e_all, s_all, Act.Exp)
            w_all = psc.tile([P, NT], dt, tag="w")
            rowsum = psc.tile([P, 1], f32, tag="rs")
            nc.vector.scalar_tensor_tensor(
                out=w_all,
                in0=e_all,
                scalar=1.0,
                in1=mask_sb[:, ts(b, NT)],
                op0=Alu.mult,
                op1=Alu.mult,
                accum_out=rowsum,
            )
            den_ps = ps_o.tile([1, 1], f32, tag="den")
            nc.tensor.matmul(den_ps, rowsum, ones_col, start=True, stop=True)
            rden = psc.tile([1, 1], f32, tag="rden")
            nc.vector.reciprocal(rden, den_ps)

            ohs = [
                ps_o.tile([1, HW], f32, tag=f"o{h}", name=f"oh{h}") for h in range(NH)
            ]
            for t in range(NT):
                ft = fts[t // CF]
                ibase = (t % CF) * RNN
                for h in range(NH):
                    nc.tensor.matmul(
                        ohs[h],
                        w_all[:, t : t + 1],
                        ft[:, ibase + h * HW : ibase + (h + 1) * HW],
                        start=(t == 0),
                        stop=(t == NT - 1),
                    )
            out_sb = pout.tile([1, RNN], f32, tag="outrow")
            for h in range(NH):
                nc.scalar.activation(
                    out_sb[:, ts(h, HW)], ohs[h], Act.Copy, scale=rden
                )
            nc.sync.dma_start(out_t[b : b + 1, :], out_sb)

    nc.compile()
    return nc


_NC_CACHE = {}


def _get_nc():
    if "nc" not in _NC_CACHE:
        _NC_CACHE["nc"] = build_nc()
    return _NC_CACHE["nc"]


def build_in_maps(h, att_feats, p_att_feats, att_masks, W_h, b_h, w_a):
    h = np.asarray(h, dtype=np.float32)
    W_h = np.asarray(W_h, dtype=np.float32)
    b_h = np.asarray(b_h, dtype=np.float32)
    w_a = np.asarray(w_a, dtype=np.float32)
    NT = S // 128
    WhT = np.ascontiguousarray(W_h.T)
    bh = b_h.reshape(1, HID)
    wa = w_a.reshape(1, HID)
    in_maps = []
    for c in range(N_CORES):
        sl = slice(c * BL, (c + 1) * BL)
        mc = (
            np.asarray(att_masks[sl])
            .astype(np.float32)
            .reshape(BL, NT, 128)
            .transpose(0, 2, 1)
        )
        in_maps.append(
            {
                "p": np.ascontiguousarray(np.asarray(p_att_feats[sl]).astype(DT_NP)),
                "f": np.ascontiguousarray(np.asarray(att_feats[sl]).astype(DT_NP)),
                "hT": np.ascontiguousarray(h[sl].T),
                "WhT": WhT,
                "bh": bh,
                "wa": wa,
                "maskc": np.ascontiguousarray(mc),
            }
        )
    return in_maps


def run(in_maps, trace=False, **kwargs):
    nc = _get_nc()
    return run_bass_kernel_spmd(
        nc, in_maps, core_ids=list(range(N_CORES)), trace=trace, **kwargs
    )


def kernel(h, att_feats, p_att_feats, att_masks, W_h, b_h, w_a, b_a=None):
    # b_a shifts every score equally; softmax normalization cancels it.
    in_maps = build_in_maps(h, att_feats, p_att_feats, att_masks, W_h, b_h, w_a)
    res = run(in_maps, trace=False)
    return np.concatenate([r["out"] for r in res.results], axis=0)
